# revision 17
# baseline (speedup 1.0000x reference)
"""Trainium2 Bass kernel for nn_BatchSparseSetConv.

Math: for each (batch b, query q, key k) the reference computes a 4-layer
ReLU MLP on the scalar a = |pos_k - x_q| (plus a one-hot channel embedding
of key k), giving a pairwise weight w = |MLP(a, ch_k)| * [a < 0.25], then
channel-wise normalized weighted sums of values.

Key identity used here: for a fixed channel c, f_c(a) = MLP(a, c) is an
exact piecewise-linear function of a. We extract its breakpoints on the
host and evaluate on device as a single hidden layer:

    f_c(a) = alpha_c + sum_j delta_cj * relu(a - t_cj)      (exact)

with at most ~5 knots per channel for typical weights. Per pair the device
does: one tiny replication matmul, one fused bias+ReLU, one knot-contraction
matmul, one |.+alpha| activation, one mask multiply -- then channel sums
become matmuls against (padded) one-hot matrices.

Sharding: data-parallel over batch, one batch per NeuronCore (B=8, 8 cores).
Each core's output is [OUT, Q]; the host transposes/stacks to [B, Q, OUT].
"""

import numpy as np

import concourse.bass as bass
import concourse.mybir as mybir
import concourse.tile as tile
from concourse import bacc
from concourse.bass_utils import run_bass_kernel_spmd

B, Q, K, C, H, OUT = 8, 1024, 1024, 16, 16, 32
WINDOW = 0.25
QT = 512
NQT = Q // QT
N_CORES = 8

F32 = mybir.dt.float32
F16 = mybir.dt.float16
AF = mybir.ActivationFunctionType
ALU = mybir.AluOpType

# chunk types: (slots_per_key, keys_per_chunk); all chunks span 32 key rows
# and 128 slot partitions. Keys whose channel needs J knots use the smallest
# type with slots_per_key >= J.
CHUNK_TYPES = [(4, 32), (5, 25), (8, 16), (16, 8)]
DEAD_POS = 9.0  # pos for padding key slots: a >= WINDOW always -> masked out


# ----------------------------------------------------------------------------
# host-side PWL extraction (exact, float64)
# ----------------------------------------------------------------------------

def _channel_pwl(W0, b0, W1, b1, W2, b2, W3, b3, c, lo=0.0, hi=WINDOW):
    """Exact breakpoints of f_c on [lo, hi): returns (t[J], delta[J], alpha).

    f_c(a) = alpha + sum_j delta[j] * relu(a - t[j]) for a in [lo, hi),
    and linear extrapolation beyond hi (masked out downstream).
    """
    W0c = W0.astype(np.float64)
    c0 = W0c[:, 1 + c] + b0.astype(np.float64)
    w0 = W0c[:, 0]
    W1c, b1c = W1.astype(np.float64), b1.astype(np.float64)
    W2c, b2c = W2.astype(np.float64), b2.astype(np.float64)
    W3c, b3c = W3.astype(np.float64), b3.astype(np.float64)

    def h1(a):
        return np.maximum(0.0, np.outer(a, w0) + c0)

    def pre2(a):
        return h1(a) @ W1c.T + b1c

    def pre3(a):
        return np.maximum(0.0, pre2(a)) @ W2c.T + b2c

    def f(a):
        return (np.maximum(0.0, pre3(a)) @ W3c.T + b3c)[:, 0]

    knots = {float(lo), float(hi)}

    def add_crossings(fn):
        ks = np.array(sorted(knots))
        v = fn(ks)
        if v.ndim == 1:
            v = v[:, None]
        for i in range(v.shape[1]):
            vi = v[:, i]
            for j in range(len(ks) - 1):
                va, vb = vi[j], vi[j + 1]
                if (va < 0) != (vb < 0) and vb != va:
                    t = ks[j] + (ks[j + 1] - ks[j]) * (-va) / (vb - va)
                    if lo < t < hi:
                        knots.add(float(t))

    add_crossings(lambda a: np.outer(a, w0) + c0)
    add_crossings(pre2)
    add_crossings(pre3)

    ks = np.array(sorted(knots))
    fv = f(ks)
    slopes = np.diff(fv) / np.diff(ks)
    t = ks[:-1].copy()  # t[0] == lo == 0
    delta = np.empty_like(slopes)
    delta[0] = slopes[0]
    delta[1:] = np.diff(slopes)
    alpha = fv[0]
    # drop numerically-zero knots to shrink J
    keep = np.abs(delta) > 1e-300
    keep[0] = True
    return t[keep], delta[keep], float(alpha)


def _all_pwl(W0, b0, W1, b1, W2, b2, W3, b3):
    ts, ds, al = [], [], []
    for c in range(C):
        t, d, a = _channel_pwl(W0, b0, W1, b1, W2, b2, W3, b3, c)
        if len(t) > 16:
            # pathological fallback: merge the smallest-|delta| knots
            order = np.argsort(np.abs(d[1:]))[::-1]
            keep = np.sort(np.concatenate([[0], 1 + order[:15]]))
            t, d = t[keep], d[keep]
        ts.append(t)
        ds.append(d)
        al.append(a)
    return ts, ds, al


# ----------------------------------------------------------------------------
# host-side packing
# ----------------------------------------------------------------------------

def _plan_chunks(ch, Js):
    """Assign keys (by channel J) to chunks. Returns list of
    (type_idx, key_index_list) with len(keys) <= keys_per_chunk."""
    type_of_key = np.zeros(len(ch), np.int32)
    for ti, (spk, _) in enumerate(CHUNK_TYPES):
        pass
    chunks = []
    for ti, (spk, cap) in enumerate(CHUNK_TYPES):
        lo = 0 if ti == 0 else CHUNK_TYPES[ti - 1][0]
        sel = [k for k in range(len(ch)) if lo < Js[ch[k]] <= spk]
        for i in range(0, len(sel), cap):
            chunks.append((ti, sel[i:i + cap]))
    return chunks


def _exp_patterns():
    """Constant expansion lhsT per chunk type: [32, 128] each, concatenated
    to [128, 128 * ntypes] f16 with the pattern replicated at each 32-aligned
    partition base (matmul requires lhsT and rhs on the same partitions).
    expT[k, s] = 1 iff slot s belongs to key k."""
    pats = []
    for spk, cap in CHUNK_TYPES:
        e = np.zeros((32, 128), np.float16)
        for s in range(spk * cap):
            e[s // spk, s] = 1.0
        pats.append(np.tile(e, (4, 1)))
    return np.concatenate(pats, axis=1)


# ----------------------------------------------------------------------------
# device program
# ----------------------------------------------------------------------------

def _build_program(nchunk, typeq):
    """Build + compile the SPMD single-core program. typeq (chunk type per
    chunk index) is baked into the instruction stream (same for all cores)."""
    ng = nchunk // 4
    nc = bacc.Bacc("TRN2", target_bir_lowering=False, debug=False)

    d_qrow = nc.dram_tensor("qrow", [1, Q], F32, kind="ExternalInput")
    d_posq = nc.dram_tensor("posq", [128, ng], F32, kind="ExternalInput")
    d_alphaq = nc.dram_tensor("alphaq", [128, ng], F32, kind="ExternalInput")
    d_knotsq = nc.dram_tensor("knotsq", [128, nchunk], F32, kind="ExternalInput")
    d_deltaq = nc.dram_tensor("deltaq", [128, 32 * nchunk], F16, kind="ExternalInput")
    d_ohq = nc.dram_tensor("ohq", [128, 16 * ng], F16, kind="ExternalInput")
    d_ohvq = nc.dram_tensor("ohvq", [128, 16 * ng], F16, kind="ExternalInput")
    d_expq = nc.dram_tensor("expq", [128, 128 * len(CHUNK_TYPES)], F16, kind="ExternalInput")
    d_sigp = nc.dram_tensor("sigp", [16, 2], F32, kind="ExternalInput")
    d_wrT = nc.dram_tensor("wrT", [16, 64], F32, kind="ExternalInput")
    d_brq = nc.dram_tensor("brq", [32, 1], F32, kind="ExternalInput")
    d_out = nc.dram_tensor("out", [32, Q], F32, kind="ExternalOutput")

    with tile.TileContext(nc) as tc:
        with tc.tile_pool(name="params", bufs=1) as params, \
             tc.tile_pool(name="qrep_p", bufs=2) as qrep_pool, \
             tc.tile_pool(name="a16_p", bufs=3) as a16_pool, \
             tc.tile_pool(name="a32_p", bufs=3) as a32_pool, \
             tc.tile_pool(name="m16_p", bufs=3) as m16_pool, \
             tc.tile_pool(name="u16_p", bufs=4) as u16_pool, \
             tc.tile_pool(name="wt_p", bufs=2) as wt_pool, \
             tc.tile_pool(name="w_p", bufs=3) as w_pool, \
             tc.tile_pool(name="epi_p", bufs=2) as epi_pool, \
             tc.tile_pool(name="ups", bufs=3, space="PSUM") as ups_pool, \
             tc.tile_pool(name="wps", bufs=2, space="PSUM") as wps_pool, \
             tc.tile_pool(name="dps", bufs=1, space="PSUM") as dps_pool, \
             tc.tile_pool(name="tps", bufs=1, space="PSUM") as tps_pool, \
             tc.tile_pool(name="eps", bufs=1, space="PSUM") as eps_pool:
            # PSUM bank budget (8 banks of [128 x 512 f32]):
            #   ups 3 + wps 2 + dps 1 + tps 1 + eps 1 = 8



            def ptile(shape, dtype, dram):
                t = params.tile(shape, dtype, tag=dram.name)
                nc.sync.dma_start(out=t[:], in_=dram.ap())
                return t

            qrow_sb = ptile([1, Q], F32, d_qrow)
            posq_sb = ptile([128, ng], F32, d_posq)
            alphaq_sb = ptile([128, ng], F32, d_alphaq)
            knotsq_sb = ptile([128, nchunk], F32, d_knotsq)
            deltaq_sb = ptile([128, 32 * nchunk], F16, d_deltaq)
            ohq_sb = ptile([128, 16 * ng], F16, d_ohq)
            ohvq_sb = ptile([128, 16 * ng], F16, d_ohvq)
            expq_sb = ptile([128, 128 * len(CHUNK_TYPES)], F16, d_expq)
            sigp_sb = ptile([16, 2], F32, d_sigp)
            # wrT layout: [:, :32] = Wr[:, :16].T (targets rows),
            #             [:, 32:] = Wr[:, 16:].T (dens rows)
            wrT_sb = ptile([16, 64], F32, d_wrT)
            brq_sb = ptile([32, 1], F32, d_brq)

            ones_sb = params.tile([1, 128], F32, tag="ones")
            nc.gpsimd.memset(ones_sb[:], 1.0)

            for qt in range(NQT):
                qs = qt * QT
                # Qrep[p, q] = queries[qs + q] for all 128 partitions
                qrep_ps = ups_pool.tile([128, QT], F32, tag="ups")
                nc.tensor.matmul(qrep_ps[:], lhsT=ones_sb[:],
                                 rhs=qrow_sb[:, qs:qs + QT], start=True, stop=True)
                qrep = qrep_pool.tile([128, QT], F32, tag="qrep")
                nc.scalar.copy(qrep[:], qrep_ps[:])

                den_ps = dps_pool.tile([16, QT], F32, tag="den")
                tnum_ps = tps_pool.tile([16, QT], F32, tag="tnum")

                for g in range(ng):
                    a32 = a32_pool.tile([128, QT], F32, tag="a32")
                    nc.scalar.activation(a32[:], qrep[:], AF.Abs,
                                         bias=posq_sb[:, g:g + 1], scale=-1.0)
                    a16 = a16_pool.tile([128, QT], F16, tag="a16")
                    nc.vector.tensor_copy(a16[:], a32[:])
                    m16 = m16_pool.tile([128, QT], F16, tag="m16")
                    nc.vector.tensor_scalar(m16[:], a32[:], WINDOW, None, ALU.is_lt)

                    w_ps = wps_pool.tile([128, QT], F32, tag="wps")
                    for c4 in range(4):
                        ci = g * 4 + c4
                        ti = int(typeq[ci])
                        u_ps = ups_pool.tile([128, QT], F32, tag="ups")
                        nc.tensor.matmul(
                            u_ps[:],
                            lhsT=expq_sb[32 * c4:32 * (c4 + 1),
                                         128 * ti:128 * (ti + 1)],
                            rhs=a16[32 * c4:32 * (c4 + 1), :],
                            start=True, stop=True,
                            tile_position=(32 * c4, 0))
                        u16 = u16_pool.tile([128, QT], F16, tag="u16")
                        if ci % 2 == 0:
                            nc.scalar.activation(u16[:], u_ps[:], AF.Relu,
                                                 bias=knotsq_sb[:, ci:ci + 1])
                        else:
                            nc.vector.tensor_scalar(u16[:], u_ps[:],
                                                    knotsq_sb[:, ci:ci + 1], 0.0,
                                                    ALU.add, ALU.max)
                        nc.tensor.matmul(
                            w_ps[32 * c4:32 * (c4 + 1), :],
                            lhsT=deltaq_sb[:, 32 * ci:32 * (ci + 1)],
                            rhs=u16[:], start=True, stop=True,
                            tile_position=(0, 32 * c4))

                    wt16 = wt_pool.tile([128, QT], F16, tag="wt")
                    nc.scalar.activation(wt16[:], w_ps[:], AF.Abs,
                                         bias=alphaq_sb[:, g:g + 1])
                    w16 = w_pool.tile([128, QT], F16, tag="w")
                    nc.vector.tensor_tensor(w16[:], wt16[:], m16[:], ALU.mult)

                    nc.tensor.matmul(den_ps[:], lhsT=ohq_sb[:, 16 * g:16 * (g + 1)],
                                     rhs=w16[:], start=(g == 0), stop=(g == ng - 1))
                    nc.tensor.matmul(tnum_ps[:], lhsT=ohvq_sb[:, 16 * g:16 * (g + 1)],
                                     rhs=w16[:], start=(g == 0), stop=(g == ng - 1))

                # epilogue for this q-tile
                den_sb = epi_pool.tile([16, QT], F32, tag="den_sb")
                nc.vector.tensor_scalar(den_sb[:], den_ps[:], 1e-5, None, ALU.add)
                rec = epi_pool.tile([16, QT], F32, tag="rec")
                scr = epi_pool.tile([16, QT], F32, tag="scr")
                nc.vector.reciprocal_approx_accurate(rec[:], den_sb[:], scr[:])
                tnum_sb = epi_pool.tile([16, QT], F32, tag="tnum_sb")
                nc.scalar.copy(tnum_sb[:], tnum_ps[:])

                targets = epi_pool.tile([16, QT], F32, tag="targets")
                nc.vector.tensor_tensor(targets[:], tnum_sb[:], rec[:], ALU.mult)
                dens = epi_pool.tile([16, QT], F32, tag="dens")
                nc.scalar.activation(dens[:], den_ps[:], AF.Sigmoid,
                                     bias=sigp_sb[:, 1:2], scale=sigp_sb[:, 0:1])

                out_ps = eps_pool.tile([32, QT], F32, tag="eps")
                nc.tensor.matmul(out_ps[:], lhsT=wrT_sb[:, 0:32], rhs=targets[:],
                                 start=True, stop=False)
                nc.tensor.matmul(out_ps[:], lhsT=wrT_sb[:, 32:64], rhs=dens[:],
                                 start=False, stop=True)
                out_sb = epi_pool.tile([32, QT], F32, tag="out_sb")
                nc.scalar.activation(out_sb[:], out_ps[:], AF.Identity,
                                     bias=brq_sb[:])
                nc.sync.dma_start(out=d_out.ap()[:, qs:qs + QT], in_=out_sb[:])

    nc.compile()
    return nc


_PROGRAM_CACHE = {}

LAST_EXEC_TIME_NS = None


def _ensure_ntff_hook():
    """The agent image's antenv lacks axon_hooks; synthesize it so
    run_bass_kernel_spmd(trace=True) can NTFF-profile via libaxon_pjrt.so."""
    import sys
    import types
    import ctypes
    import contextlib
    try:
        import antenv.axon_hooks  # noqa: F401
        return True
    except ImportError:
        pass
    so_path = "/opt/axon/libaxon_pjrt.so"
    try:
        lib = ctypes.CDLL(so_path)
    except OSError:
        return False
    if not hasattr(lib, "axon_start_nrt_profile"):
        return False
    lib.axon_start_nrt_profile.argtypes = [ctypes.POINTER(ctypes.c_int64),
                                           ctypes.c_size_t]
    lib.axon_start_nrt_profile.restype = ctypes.c_int64
    lib.axon_stop_nrt_profile.argtypes = [ctypes.c_char_p]
    lib.axon_stop_nrt_profile.restype = ctypes.c_int64

    @contextlib.contextmanager
    def _hook(output_dir, device_ids):
        import jax
        jax.devices()
        if device_ids:
            ids = (ctypes.c_int64 * len(device_ids))(*device_ids)
            rc = lib.axon_start_nrt_profile(ids, len(device_ids))
        else:
            rc = lib.axon_start_nrt_profile(None, 0)
        if rc != 0:
            raise RuntimeError(f"axon_start_nrt_profile rc={rc}")
        try:
            yield
        finally:
            n = lib.axon_stop_nrt_profile(str(output_dir).encode())
            print(f"profile: {n} file(s) written to {output_dir}")

    mod = types.ModuleType("antenv.axon_hooks")
    mod.get_axon_ntff_profile_hook = lambda: _hook
    mod.set_axon_ntff_profile_hook = lambda h: None
    import antenv
    antenv.axon_hooks = mod
    sys.modules["antenv.axon_hooks"] = mod
    return True


def _get_program(nchunk, typeq):
    key = (nchunk, tuple(int(t) for t in typeq))
    if key not in _PROGRAM_CACHE:
        _PROGRAM_CACHE[key] = _build_program(nchunk, typeq)
    return _PROGRAM_CACHE[key]


# ----------------------------------------------------------------------------
# entry point
# ----------------------------------------------------------------------------

def kernel(trace=False, **inputs):
    global LAST_EXEC_TIME_NS
    keys_in = np.asarray(inputs["keys_in"], np.float32)
    queries = np.asarray(inputs["queries"], np.float32)
    values = np.asarray(inputs["values"], np.float32)
    W0 = np.asarray(inputs["W0"], np.float32)
    b0 = np.asarray(inputs["b0"], np.float32)
    W1 = np.asarray(inputs["W1"], np.float32)
    b1 = np.asarray(inputs["b1"], np.float32)
    W2 = np.asarray(inputs["W2"], np.float32)
    b2 = np.asarray(inputs["b2"], np.float32)
    W3 = np.asarray(inputs["W3"], np.float32)
    b3 = np.asarray(inputs["b3"], np.float32)
    Wd = np.asarray(inputs["Wd"], np.float32)
    bd = np.asarray(inputs["bd"], np.float32)
    Wr = np.asarray(inputs["Wr"], np.float32)
    br = np.asarray(inputs["br"], np.float32)

    pwl = _all_pwl(W0, b0, W1, b1, W2, b2, W3, b3)
    Js = [len(t) for t in pwl[0]]

    # plan chunks per core, find common padded chunk count and type layout
    per_core = []
    for b in range(B):
        ch = keys_in[b, :, 0].astype(np.int32)
        per_core.append(_plan_chunks(ch, Js))
    nchunk = max(len(cks) for cks in per_core)
    nchunk = (nchunk + 3) // 4 * 4

    # chunk TYPE sequence must be identical across cores (single program).
    # Pad every core's chunk list to the same per-type counts.
    ntypes_needed = []
    for ti in range(len(CHUNK_TYPES)):
        ntypes_needed.append(max(sum(1 for t, _ in cks if t == ti)
                                 for cks in per_core))
    total = sum(ntypes_needed)
    pad = nchunk - total
    if pad < 0:
        nchunk = (total + 3) // 4 * 4
        pad = nchunk - total
    ntypes_needed[0] += pad
    typeq = np.concatenate([np.full(n, ti, np.int32)
                            for ti, n in enumerate(ntypes_needed)])

    sig_scale = np.float32(0.1) * Wd[0, 0]
    sig_bias = bd[0] - Wd[0, 0]
    sigp = np.stack([np.full(16, sig_scale, np.float32),
                     np.full(16, sig_bias, np.float32)], axis=1)
    expq = _exp_patterns()
    wrT = np.concatenate([Wr[:, :16].T, Wr[:, 16:].T],
                         axis=1).astype(np.float32)
    brq = br.astype(np.float32)[:, None]

    in_maps = []
    for b in range(B):
        packed, _ = _pack_core_typed(keys_in[b], queries[b], values[b], pwl,
                                     nchunk, typeq)
        packed.update(sigp=sigp, wrT=wrT, brq=brq, expq=expq)
        in_maps.append(packed)

    nc = _get_program(nchunk, typeq)
    if trace:
        trace = _ensure_ntff_hook()
    res = run_bass_kernel_spmd(nc, in_maps, list(range(N_CORES)), trace=trace)
    if trace:
        LAST_EXEC_TIME_NS = res.exec_time_ns
    out = np.stack([np.ascontiguousarray(res.results[i]["out"].T)
                    for i in range(N_CORES)], axis=0)
    return out.astype(np.float32)


def _pack_core_typed(keys_in_b, queries_b, values_b, pwl, nchunk, typeq):
    """Pack one core's inputs honoring a fixed chunk-type sequence typeq."""
    ts, ds, al = pwl
    Js = [len(t) for t in ts]
    ch = keys_in_b[:, 0].astype(np.int32)
    pos = keys_in_b[:, 1].astype(np.float32)
    vsel = values_b[np.arange(K), ch].astype(np.float32)

    chunks = _plan_chunks(ch, Js)
    # slot chunks into the typeq sequence
    slots_by_type = {ti: [i for i, t in enumerate(typeq) if t == ti]
                     for ti in range(len(CHUNK_TYPES))}
    used = {ti: 0 for ti in slots_by_type}
    placed = [None] * len(typeq)
    for (ti, keys) in chunks:
        idx = slots_by_type[ti][used[ti]]
        used[ti] += 1
        placed[idx] = (ti, keys)

    ng = nchunk // 4
    qrow = queries_b[:, 0].astype(np.float32)[None, :]
    posq = np.full((128, ng), DEAD_POS, np.float32)
    alphaq = np.zeros((128, ng), np.float32)
    knotsq = np.full((128, nchunk), -9.0, np.float32)
    deltaq = np.zeros((128, 32 * nchunk), np.float16)
    ohq = np.zeros((128, 16 * ng), np.float16)
    ohvq = np.zeros((128, 16 * ng), np.float16)

    for ci in range(nchunk):
        if placed[ci] is None:
            continue
        ti, keys = placed[ci]
        spk, cap = CHUNK_TYPES[ti]
        g, c4 = ci // 4, ci % 4
        for k_i, k in enumerate(keys):
            c = ch[k]
            p = 32 * c4 + k_i
            posq[p, g] = pos[k]
            alphaq[p, g] = al[c]
            ohq[p, 16 * g + c] = np.float16(1.0)
            ohvq[p, 16 * g + c] = np.float16(vsel[k])
            J = Js[c]
            s0 = spk * k_i
            knotsq[s0:s0 + J, ci] = -ts[c].astype(np.float32)
            deltaq[s0:s0 + J, 32 * ci + k_i] = ds[c].astype(np.float16)
    return dict(qrow=qrow, posq=posq, alphaq=alphaq, knotsq=knotsq,
                deltaq=deltaq, ohq=ohq, ohvq=ohvq), typeq


# revision 26
# speedup vs baseline: 1.4002x; 1.4002x over previous
"""Trainium2 Bass kernel for nn_BatchSparseSetConv.

Math: for each (batch b, query q, key k) the reference computes a 4-layer
ReLU MLP on the scalar a = |pos_k - x_q| plus a one-hot channel embedding,
giving a pairwise weight w = |MLP(a, ch_k)| * [a < 0.25], then channel-wise
normalized weighted sums of values.

Key identity: for fixed channel c, f_c(a) = MLP(a, c) is an exact
piecewise-linear function of a. Host extracts its breakpoints and the device
evaluates

    f_c(a) = alpha_c + beta_c * a + sum_{j>=1} delta_cj * relu(a - t_cj)

exactly. The linear part (alpha, beta) is folded into a per-group diagonal
matmul + the |.|-activation bias; only interior knots need the
expand->relu->contract path. Keys are packed into variable-size "chunks"
(128/64/32/... keys depending on knot count) so each chunk fills the
128-partition slot budget.

Sharding: data-parallel over batch, one batch per core (B=8 = 8 cores).
Device output is [OUT, Q] per core; host transposes/stacks.
"""

import numpy as np

import concourse.bass as bass
import concourse.mybir as mybir
import concourse.tile as tile
from concourse import bacc
from concourse.bass_utils import run_bass_kernel_spmd

B, Q, K, C, H, OUT = 8, 1024, 1024, 16, 16, 32
WINDOW = 0.25
QT = 512
NQT = Q // QT
N_CORES = 8

F32 = mybir.dt.float32
F16 = mybir.dt.float16
AF = mybir.ActivationFunctionType
ALU = mybir.AluOpType

DEAD_POS = 9.0  # pos for padding key rows: a >= WINDOW always -> masked out


# ----------------------------------------------------------------------------
# host-side PWL extraction (exact, float64)
# ----------------------------------------------------------------------------

def _channel_pwl(W0, b0, W1, b1, W2, b2, W3, b3, c, lo=0.0, hi=WINDOW):
    """Exact PWL of f_c on [lo, hi): returns (t[J], delta[J], alpha) where
    f_c(a) = alpha + sum_j delta[j]*relu(a - t[j]), t[0] == 0."""
    W0c = W0.astype(np.float64)
    c0 = W0c[:, 1 + c] + b0.astype(np.float64)
    w0 = W0c[:, 0]
    W1c, b1c = W1.astype(np.float64), b1.astype(np.float64)
    W2c, b2c = W2.astype(np.float64), b2.astype(np.float64)
    W3c, b3c = W3.astype(np.float64), b3.astype(np.float64)

    def h1(a):
        return np.maximum(0.0, np.outer(a, w0) + c0)

    def pre2(a):
        return h1(a) @ W1c.T + b1c

    def pre3(a):
        return np.maximum(0.0, pre2(a)) @ W2c.T + b2c

    def f(a):
        return (np.maximum(0.0, pre3(a)) @ W3c.T + b3c)[:, 0]

    knots = {float(lo), float(hi)}

    def add_crossings(fn):
        ks = np.array(sorted(knots))
        v = fn(ks)
        if v.ndim == 1:
            v = v[:, None]
        for i in range(v.shape[1]):
            vi = v[:, i]
            for j in range(len(ks) - 1):
                va, vb = vi[j], vi[j + 1]
                if (va < 0) != (vb < 0) and vb != va:
                    t = ks[j] + (ks[j + 1] - ks[j]) * (-va) / (vb - va)
                    if lo < t < hi:
                        knots.add(float(t))

    add_crossings(lambda a: np.outer(a, w0) + c0)
    add_crossings(pre2)
    add_crossings(pre3)

    ks = np.array(sorted(knots))
    fv = f(ks)
    slopes = np.diff(fv) / np.diff(ks)
    t = ks[:-1].copy()
    delta = np.empty_like(slopes)
    delta[0] = slopes[0]
    delta[1:] = np.diff(slopes)
    keep = np.abs(delta) > 1e-300
    keep[0] = True
    return t[keep], delta[keep], float(fv[0])


def _all_pwl(W0, b0, W1, b1, W2, b2, W3, b3):
    ts, ds, al = [], [], []
    for c in range(C):
        t, d, a = _channel_pwl(W0, b0, W1, b1, W2, b2, W3, b3, c)
        if len(t) > 16:
            order = np.argsort(np.abs(d[1:]))[::-1]
            keep = np.sort(np.concatenate([[0], 1 + order[:15]]))
            t, d = t[keep], d[keep]
        ts.append(t)
        ds.append(d)
        al.append(a)
    return ts, ds, al


# ----------------------------------------------------------------------------
# chunk structure planning (shared across cores; sized by max class counts)
# ----------------------------------------------------------------------------

def _keys_per_chunk(spk):
    """#keys a chunk of spk interior-knots-per-key can hold (<=128 slots)."""
    if spk <= 1:
        return 128
    if spk == 2:
        return 64
    if spk <= 4:
        return 32
    return min(32, 128 // spk)


def _chunk_rows(nkeys):
    """Rows claimed in the group (alignment granule)."""
    return max(32, nkeys)


def plan_structure(max_count_by_spk, max_linear=0):
    """Given max (over cores) #keys per spk class (spk>=1), return the static
    chunk/group structure: list of chunks (spk, g, kb, nkeys) and NG."""
    chunks = []
    for spk in sorted(max_count_by_spk, reverse=True):
        n = max_count_by_spk[spk]
        cap = _keys_per_chunk(spk)
        while n > 0:
            take = min(cap, n)
            chunks.append({"spk": spk, "nkeys": cap, "used": take})
            n -= take
    # rows demand: chunk rows + linear keys share leftover; compute groups
    # first-fit-decreasing by row footprint
    chunks.sort(key=lambda c: -_chunk_rows(c["nkeys"]))
    groups = []  # each: free row map as list of (base, size) with 32-granule

    def alloc(rows):
        need = rows
        for gi, free in enumerate(groups):
            # find aligned run of `need` rows (need is mult of 32 or < 32)
            granule = max(need, 32)
            for base in range(0, 128, granule):
                span = [b for b in range(base, base + granule, 32)]
                if all(b in free for b in span):
                    for b in span:
                        free.remove(b)
                    return gi, base
        groups.append({0, 32, 64, 96})
        return alloc(rows)

    for ck in chunks:
        g, kb = alloc(_chunk_rows(ck["nkeys"]))
        ck["g"], ck["kb"] = g, kb
    while sum(len(f) for f in groups) * 32 < max_linear:
        groups.append({0, 32, 64, 96})
    ng = len(groups)
    free_rows = sum(len(f) for f in groups) * 32
    return chunks, ng, groups, free_rows


# ----------------------------------------------------------------------------
# per-core packing
# ----------------------------------------------------------------------------

def pack_core(keys_in_b, queries_b, values_b, pwl, structure):
    ts, ds, al = pwl
    chunks, ng, groups_free, _ = structure
    nchunk = len(chunks)
    ch = keys_in_b[:, 0].astype(np.int32)
    pos = keys_in_b[:, 1].astype(np.float32)
    vsel = values_b[np.arange(K), ch].astype(np.float32)

    spk_of_key = np.array([len(ts[c]) - 1 for c in ch], np.int32)

    qrow = queries_b[:, 0].astype(np.float32)[None, :]
    posq = np.full((128, ng), DEAD_POS, np.float32)
    alphaq = np.zeros((128, ng), np.float32)
    knotsq = np.full((128, nchunk), -9.0, np.float32)
    deltaq = np.zeros((128, 128 * nchunk), np.float16)
    expqc = np.zeros((128, 128 * nchunk), np.float16)
    ddiag = np.zeros((128, 128 * ng), np.float16)
    ohov = np.zeros((128, 48 * ng), np.float16)

    def place_key(k, g, row):
        c = ch[k]
        posq[row, g] = pos[k]
        alphaq[row, g] = al[c]
        ddiag[row, 128 * g + row] = np.float16(ds[c][0])
        ohov[row, 48 * g + c] = np.float16(1.0)
        ohov[row, 48 * g + 32 + c] = np.float16(vsel[k])

    # nonlinear keys -> chunks of matching spk
    by_spk = {}
    for k in range(K):
        by_spk.setdefault(int(spk_of_key[k]), []).append(k)
    linear_keys = by_spk.pop(0, [])
    linear_keys += by_spk.pop(-1, []) if -1 in by_spk else []

    for ci, ck in enumerate(chunks):
        spk, g, kb, cap = ck["spk"], ck["g"], ck["kb"], ck["nkeys"]
        pool = by_spk.get(spk, [])
        take = pool[:cap]
        by_spk[spk] = pool[cap:]
        for k_i, k in enumerate(take):
            c = ch[k]
            row = kb + k_i
            place_key(k, g, row)
            t_int = ts[c][1:]
            d_int = ds[c][1:]
            s0 = spk * k_i
            knotsq[s0:s0 + spk, ci] = -t_int.astype(np.float32)
            # fin matmuls write the full 128-row group; key lands at row kb+k_i
            deltaq[s0:s0 + spk, 128 * ci + kb + k_i] = d_int.astype(np.float16)
            expqc[kb + k_i, 128 * ci + s0:128 * ci + s0 + spk] = np.float16(1.0)
    for spk, rem in by_spk.items():
        assert not rem, f"unplaced keys of class {spk}"

    # linear keys fill leftover rows
    free_slots = []
    for g, free in enumerate(groups_free):
        for b in sorted(free):
            for r in range(b, b + 32):
                free_slots.append((g, r))
    # also rows inside chunks beyond 'used'... chunks may be partially filled
    # per-core; those rows already default to dead (posq=DEAD_POS).
    assert len(free_slots) >= len(linear_keys), (len(free_slots), len(linear_keys))
    for (g, r), k in zip(free_slots, linear_keys):
        place_key(k, g, r)

    return dict(qrow=qrow, posq=posq, alphaq=alphaq, knotsq=knotsq,
                deltaq=deltaq, expqc=expqc, ddiag=ddiag, ohov=ohov)


# ----------------------------------------------------------------------------
# device program
# ----------------------------------------------------------------------------

def _build_program(structure):
    chunks, ng, _, _ = structure
    nchunk = len(chunks)
    nc = bacc.Bacc("TRN2", target_bir_lowering=False, debug=False)

    d_qrow = nc.dram_tensor("qrow", [1, Q], F32, kind="ExternalInput")
    d_posq = nc.dram_tensor("posq", [128, ng], F32, kind="ExternalInput")
    d_alphaq = nc.dram_tensor("alphaq", [128, ng], F32, kind="ExternalInput")
    d_knotsq = nc.dram_tensor("knotsq", [128, nchunk], F32, kind="ExternalInput")
    d_deltaq = nc.dram_tensor("deltaq", [128, 128 * nchunk], F16, kind="ExternalInput")
    d_expqc = nc.dram_tensor("expqc", [128, 128 * nchunk], F16, kind="ExternalInput")
    d_ddiag = nc.dram_tensor("ddiag", [128, 128 * ng], F16, kind="ExternalInput")
    d_ohov = nc.dram_tensor("ohov", [128, 48 * ng], F16, kind="ExternalInput")
    d_sigp = nc.dram_tensor("sigp", [16, 2], F32, kind="ExternalInput")
    d_wrT = nc.dram_tensor("wrT", [16, 64], F32, kind="ExternalInput")
    d_brq = nc.dram_tensor("brq", [32, 1], F32, kind="ExternalInput")
    d_out = nc.dram_tensor("out", [32, Q], F32, kind="ExternalOutput")

    # chunks grouped by g for the emission loop
    by_group = [[] for _ in range(ng)]
    for ci, ck in enumerate(chunks):
        by_group[ck["g"]].append((ci, ck))

    with tile.TileContext(nc) as tc:
        with tc.tile_pool(name="params", bufs=1) as params, \
             tc.tile_pool(name="qrep_p", bufs=1) as qrep_pool, \
             tc.tile_pool(name="a16_p", bufs=2) as a16_pool, \
             tc.tile_pool(name="a32_p", bufs=2) as a32_pool, \
             tc.tile_pool(name="m16_p", bufs=2) as m16_pool, \
             tc.tile_pool(name="u16_p", bufs=3) as u16_pool, \
             tc.tile_pool(name="wt_p", bufs=2) as wt_pool, \
             tc.tile_pool(name="w_p", bufs=3) as w_pool, \
             tc.tile_pool(name="epi_p", bufs=2) as epi_pool, \
             tc.tile_pool(name="ups", bufs=2, space="PSUM") as ups_pool, \
             tc.tile_pool(name="wps", bufs=2, space="PSUM") as wps_pool, \
             tc.tile_pool(name="dps", bufs=2, space="PSUM") as dps_pool:
            # PSUM banks: ups 2x[128,1024]=4 + wps 2x[128,512]=2 +
            #             dps 2x[48,512]=2  -> 8

            def ptile(shape, dtype, dram):
                t = params.tile(shape, dtype, tag=dram.name)
                nc.sync.dma_start(out=t[:], in_=dram.ap())
                return t

            qrow_sb = ptile([1, Q], F32, d_qrow)
            posq_sb = ptile([128, ng], F32, d_posq)
            alphaq_sb = ptile([128, ng], F32, d_alphaq)
            knotsq_sb = ptile([128, nchunk], F32, d_knotsq)
            deltaq_sb = ptile([128, 128 * nchunk], F16, d_deltaq)
            expqc_sb = ptile([128, 128 * nchunk], F16, d_expqc)
            ddiag_sb = ptile([128, 128 * ng], F16, d_ddiag)
            ohov_sb = ptile([128, 48 * ng], F16, d_ohov)
            sigp_sb = ptile([16, 2], F32, d_sigp)
            wrT_sb = ptile([16, 64], F32, d_wrT)
            brq_sb = ptile([32, 1], F32, d_brq)

            ones_sb = params.tile([1, 128], F32, tag="ones")
            nc.gpsimd.memset(ones_sb[:], 1.0)

            # Qrep for the whole batch: [128, 1024]
            qrep_ps = ups_pool.tile([128, Q], F32, tag="ups")
            for qt in range(NQT):
                nc.tensor.matmul(qrep_ps[:, qt * QT:(qt + 1) * QT], lhsT=ones_sb[:],
                                 rhs=qrow_sb[:, qt * QT:(qt + 1) * QT],
                                 start=True, stop=True)
            qrep = qrep_pool.tile([128, Q], F32, tag="qrep")
            nc.scalar.copy(qrep[:], qrep_ps[:])

            dt_ps = [dps_pool.tile([48, QT], F32, tag="dt", name=f"dt_ps{qt}")
                     for qt in range(NQT)]

            relu_ct = 0
            for g in range(ng):
                cks = by_group[g]
                a32 = a32_pool.tile([128, Q], F32, tag="a32")
                nc.scalar.activation(a32[:], qrep[:], AF.Abs,
                                     bias=posq_sb[:, g:g + 1], scale=-1.0)
                a16 = a16_pool.tile([128, Q], F16, tag="a16")
                nc.vector.tensor_copy(a16[:], a32[:])
                m16 = m16_pool.tile([128, Q], F16, tag="m16")
                nc.vector.tensor_scalar(m16[:], a32[:], WINDOW, None, ALU.is_lt)

                # w_ps per q-half: diagonal (linear term) + chunk contractions
                w_ps = [wps_pool.tile([128, QT], F32, tag="wps",
                                        name=f"w_ps_g{g}q{qt}")
                            for qt in range(NQT)]
                for qt in range(NQT):
                    nc.tensor.matmul(w_ps[qt][:],
                                     lhsT=ddiag_sb[:, 128 * g:128 * (g + 1)],
                                     rhs=a16[:, qt * QT:(qt + 1) * QT],
                                     start=True, stop=(len(cks) == 0))

                u16s = []
                for ci, ck in cks:
                    kb, nk = ck["kb"], ck["nkeys"]
                    u_ps = ups_pool.tile([128, Q], F32, tag="ups")
                    for qt in range(NQT):
                        nc.tensor.matmul(
                            u_ps[:, qt * QT:(qt + 1) * QT],
                            lhsT=expqc_sb[kb:kb + nk, 128 * ci:128 * (ci + 1)],
                            rhs=a16[kb:kb + nk, qt * QT:(qt + 1) * QT],
                            start=True, stop=True,
                            tile_position=(kb if nk <= 64 else 0, 0))
                    u16 = u16_pool.tile([128, Q], F16, tag="u16")
                    if relu_ct % 2 == 0:
                        nc.scalar.activation(u16[:], u_ps[:], AF.Relu,
                                             bias=knotsq_sb[:, ci:ci + 1])
                    else:
                        nc.vector.tensor_scalar(u16[:], u_ps[:],
                                                knotsq_sb[:, ci:ci + 1], 0.0,
                                                ALU.add, ALU.max)
                    relu_ct += 1
                    u16s.append((ci, ck, u16))

                for i, (ci, ck, u16) in enumerate(u16s):
                    # full 128-key write (zero delta cols elsewhere) so the
                    # last fin's stop closes the whole accumulation group
                    last = (i == len(u16s) - 1)
                    for qt in range(NQT):
                        nc.tensor.matmul(
                            w_ps[qt][:],
                            lhsT=deltaq_sb[:, 128 * ci:128 * (ci + 1)],
                            rhs=u16[:, qt * QT:(qt + 1) * QT],
                            start=False, stop=last)

                for qt in range(NQT):
                    wt16 = wt_pool.tile([128, QT], F16, tag="wt")
                    nc.scalar.activation(wt16[:], w_ps[qt][:], AF.Abs,
                                         bias=alphaq_sb[:, g:g + 1])
                    w16 = w_pool.tile([128, QT], F16, tag="w")
                    nc.vector.tensor_tensor(w16[:], wt16[:],
                                            m16[:, qt * QT:(qt + 1) * QT], ALU.mult)
                    nc.tensor.matmul(dt_ps[qt][:],
                                     lhsT=ohov_sb[:, 48 * g:48 * (g + 1)],
                                     rhs=w16[:], start=(g == 0), stop=(g == ng - 1))

            for qt in range(NQT):
                qs = qt * QT
                den_sb = epi_pool.tile([16, QT], F32, tag="den_sb")
                nc.vector.tensor_scalar(den_sb[:], dt_ps[qt][0:16, :], 1e-5,
                                        None, ALU.add)
                rec = epi_pool.tile([16, QT], F32, tag="rec")
                scr = epi_pool.tile([16, QT], F32, tag="scr")
                nc.vector.reciprocal_approx_accurate(rec[:], den_sb[:], scr[:])
                tnum_sb = epi_pool.tile([16, QT], F32, tag="tnum_sb")
                nc.scalar.copy(tnum_sb[:], dt_ps[qt][32:48, :])

                targets = epi_pool.tile([16, QT], F32, tag="targets")
                nc.vector.tensor_tensor(targets[:], tnum_sb[:], rec[:], ALU.mult)
                dens = epi_pool.tile([16, QT], F32, tag="dens")
                nc.scalar.activation(dens[:], dt_ps[qt][0:16, :], AF.Sigmoid,
                                     bias=sigp_sb[:, 1:2], scale=sigp_sb[:, 0:1])

                out_ps = dps_pool.tile([32, QT], F32, tag="dt")
                nc.tensor.matmul(out_ps[:], lhsT=wrT_sb[:, 0:32], rhs=targets[:],
                                 start=True, stop=False)
                nc.tensor.matmul(out_ps[:], lhsT=wrT_sb[:, 32:64], rhs=dens[:],
                                 start=False, stop=True)
                out_sb = epi_pool.tile([32, QT], F32, tag="out_sb")
                nc.scalar.activation(out_sb[:], out_ps[:], AF.Identity,
                                     bias=brq_sb[:])
                nc.sync.dma_start(out=d_out.ap()[:, qs:qs + QT], in_=out_sb[:])

    nc.compile()
    return nc


_PROGRAM_CACHE = {}

LAST_EXEC_TIME_NS = None
LAST_RESULTS = None


def _ensure_ntff_hook():
    """The agent image's antenv lacks axon_hooks; synthesize it so
    run_bass_kernel_spmd(trace=True) can NTFF-profile via libaxon_pjrt.so."""
    import sys
    import types
    import ctypes
    import contextlib
    try:
        import antenv.axon_hooks  # noqa: F401
        return True
    except ImportError:
        pass
    so_path = "/opt/axon/libaxon_pjrt.so"
    try:
        lib = ctypes.CDLL(so_path)
    except OSError:
        return False
    if not hasattr(lib, "axon_start_nrt_profile"):
        return False
    lib.axon_start_nrt_profile.argtypes = [ctypes.POINTER(ctypes.c_int64),
                                           ctypes.c_size_t]
    lib.axon_start_nrt_profile.restype = ctypes.c_int64
    lib.axon_stop_nrt_profile.argtypes = [ctypes.c_char_p]
    lib.axon_stop_nrt_profile.restype = ctypes.c_int64

    @contextlib.contextmanager
    def _hook(output_dir, device_ids):
        import jax
        jax.devices()
        if device_ids:
            ids = (ctypes.c_int64 * len(device_ids))(*device_ids)
            rc = lib.axon_start_nrt_profile(ids, len(device_ids))
        else:
            rc = lib.axon_start_nrt_profile(None, 0)
        if rc != 0:
            raise RuntimeError(f"axon_start_nrt_profile rc={rc}")
        try:
            yield
        finally:
            n = lib.axon_stop_nrt_profile(str(output_dir).encode())
            print(f"profile: {n} file(s) written to {output_dir}")

    mod = types.ModuleType("antenv.axon_hooks")
    mod.get_axon_ntff_profile_hook = lambda: _hook
    mod.set_axon_ntff_profile_hook = lambda h: None
    import antenv
    antenv.axon_hooks = mod
    sys.modules["antenv.axon_hooks"] = mod
    return True


def _structure_key(structure):
    chunks, ng, _, _ = structure
    return (ng,) + tuple((c["spk"], c["g"], c["kb"], c["nkeys"]) for c in chunks)


def _get_program(structure):
    key = _structure_key(structure)
    if key not in _PROGRAM_CACHE:
        _PROGRAM_CACHE[key] = _build_program(structure)
    return _PROGRAM_CACHE[key]


# ----------------------------------------------------------------------------
# entry point
# ----------------------------------------------------------------------------

def kernel(trace=False, **inputs):
    global LAST_EXEC_TIME_NS, LAST_RESULTS
    keys_in = np.asarray(inputs["keys_in"], np.float32)
    queries = np.asarray(inputs["queries"], np.float32)
    values = np.asarray(inputs["values"], np.float32)
    W = {k: np.asarray(inputs[k], np.float32)
         for k in ["W0", "b0", "W1", "b1", "W2", "b2", "W3", "b3",
                   "Wd", "bd", "Wr", "br"]}

    pwl = _all_pwl(W["W0"], W["b0"], W["W1"], W["b1"], W["W2"], W["b2"],
                   W["W3"], W["b3"])
    ts = pwl[0]
    spk_by_channel = np.array([len(t) - 1 for t in ts], np.int32)

    # max #keys of each class over cores (spk==0 -> linear, no chunk needed)
    max_count = {}
    max_linear = 0
    for b in range(B):
        ch = keys_in[b, :, 0].astype(np.int32)
        spk = spk_by_channel[ch]
        max_linear = max(max_linear, int((spk == 0).sum()))
        for s in range(1, 17):
            n = int((spk == s).sum())
            if n:
                max_count[s] = max(max_count.get(s, 0), n)
    structure = plan_structure(max_count, max_linear)

    sig_scale = np.float32(0.1) * W["Wd"][0, 0]
    sig_bias = W["bd"][0] - W["Wd"][0, 0]
    sigp = np.stack([np.full(16, sig_scale, np.float32),
                     np.full(16, sig_bias, np.float32)], axis=1)
    Wr = W["Wr"]
    wrT = np.concatenate([Wr[:, :16].T, Wr[:, 16:].T], axis=1).astype(np.float32)
    brq = W["br"].astype(np.float32)[:, None]

    in_maps = []
    for b in range(B):
        packed = pack_core(keys_in[b], queries[b], values[b], pwl, structure)
        packed.update(sigp=sigp, wrT=wrT, brq=brq)
        in_maps.append(packed)

    nc = _get_program(structure)
    if trace:
        trace = _ensure_ntff_hook()
    res = run_bass_kernel_spmd(nc, in_maps, list(range(N_CORES)), trace=trace)
    LAST_RESULTS = res
    if trace:
        LAST_EXEC_TIME_NS = res.exec_time_ns
    out = np.stack([np.ascontiguousarray(res.results[i]["out"].T)
                    for i in range(N_CORES)], axis=0)
    return out.astype(np.float32)


# revision 27
# speedup vs baseline: 1.4709x; 1.0505x over previous
"""Trainium2 Bass kernel for nn_BatchSparseSetConv.

Math: for each (batch b, query q, key k) the reference computes a 4-layer
ReLU MLP on the scalar a = |pos_k - x_q| plus a one-hot channel embedding,
giving a pairwise weight w = |MLP(a, ch_k)| * [a < 0.25], then channel-wise
normalized weighted sums of values.

Key identity: for fixed channel c, f_c(a) = MLP(a, c) is an exact
piecewise-linear function of a. Host extracts its breakpoints and the device
evaluates

    f_c(a) = alpha_c + beta_c * a + sum_{j>=1} delta_cj * relu(a - t_cj)

exactly. The linear part (alpha, beta) is folded into a per-group diagonal
matmul + the |.|-activation bias; only interior knots need the
expand->relu->contract path. Keys are packed into variable-size "chunks"
(128/64/32/... keys depending on knot count) so each chunk fills the
128-partition slot budget.

Sharding: data-parallel over batch, one batch per core (B=8 = 8 cores).
Device output is [OUT, Q] per core; host transposes/stacks.
"""

import numpy as np

import concourse.bass as bass
import concourse.mybir as mybir
import concourse.tile as tile
from concourse import bacc
from concourse.bass_utils import run_bass_kernel_spmd

B, Q, K, C, H, OUT = 8, 1024, 1024, 16, 16, 32
WINDOW = 0.25
QT = 512
NQT = Q // QT
N_CORES = 8

F32 = mybir.dt.float32
F16 = mybir.dt.float16
AF = mybir.ActivationFunctionType
ALU = mybir.AluOpType

DEAD_POS = 9.0  # pos for padding key rows: a >= WINDOW always -> masked out


# ----------------------------------------------------------------------------
# host-side PWL extraction (exact, float64)
# ----------------------------------------------------------------------------

def _channel_pwl(W0, b0, W1, b1, W2, b2, W3, b3, c, lo=0.0, hi=WINDOW):
    """Exact PWL of f_c on [lo, hi): returns (t[J], delta[J], alpha) where
    f_c(a) = alpha + sum_j delta[j]*relu(a - t[j]), t[0] == 0."""
    W0c = W0.astype(np.float64)
    c0 = W0c[:, 1 + c] + b0.astype(np.float64)
    w0 = W0c[:, 0]
    W1c, b1c = W1.astype(np.float64), b1.astype(np.float64)
    W2c, b2c = W2.astype(np.float64), b2.astype(np.float64)
    W3c, b3c = W3.astype(np.float64), b3.astype(np.float64)

    def h1(a):
        return np.maximum(0.0, np.outer(a, w0) + c0)

    def pre2(a):
        return h1(a) @ W1c.T + b1c

    def pre3(a):
        return np.maximum(0.0, pre2(a)) @ W2c.T + b2c

    def f(a):
        return (np.maximum(0.0, pre3(a)) @ W3c.T + b3c)[:, 0]

    knots = {float(lo), float(hi)}

    def add_crossings(fn):
        ks = np.array(sorted(knots))
        v = fn(ks)
        if v.ndim == 1:
            v = v[:, None]
        for i in range(v.shape[1]):
            vi = v[:, i]
            for j in range(len(ks) - 1):
                va, vb = vi[j], vi[j + 1]
                if (va < 0) != (vb < 0) and vb != va:
                    t = ks[j] + (ks[j + 1] - ks[j]) * (-va) / (vb - va)
                    if lo < t < hi:
                        knots.add(float(t))

    add_crossings(lambda a: np.outer(a, w0) + c0)
    add_crossings(pre2)
    add_crossings(pre3)

    ks = np.array(sorted(knots))
    fv = f(ks)
    slopes = np.diff(fv) / np.diff(ks)
    t = ks[:-1].copy()
    delta = np.empty_like(slopes)
    delta[0] = slopes[0]
    delta[1:] = np.diff(slopes)
    keep = np.abs(delta) > 1e-300
    keep[0] = True
    return t[keep], delta[keep], float(fv[0])


def _all_pwl(W0, b0, W1, b1, W2, b2, W3, b3):
    ts, ds, al = [], [], []
    for c in range(C):
        t, d, a = _channel_pwl(W0, b0, W1, b1, W2, b2, W3, b3, c)
        if len(t) > 16:
            order = np.argsort(np.abs(d[1:]))[::-1]
            keep = np.sort(np.concatenate([[0], 1 + order[:15]]))
            t, d = t[keep], d[keep]
        ts.append(t)
        ds.append(d)
        al.append(a)
    return ts, ds, al


# ----------------------------------------------------------------------------
# group structure planning (shared across cores; sized by max class counts)
#
# Groups of 128 key rows, sorted by spk (interior knots per key) descending.
# Group g evaluates spk_g = max-spk-in-group knot ReLUs directly on a16
# (identity slot=key mapping) and accumulates w via diagonal matmuls:
#     w = diag(beta) @ a16 + sum_j diag(delta_j) @ relu(a16 - t_j)
# ----------------------------------------------------------------------------

def plan_structure(max_count_by_spk, max_linear=0):
    """Return (row_classes, group_spk): row_classes = per-class (spk, nrows)
    run-list in descending spk order; group_spk[g] = max spk in group g."""
    runs = []
    for spk in sorted(max_count_by_spk, reverse=True):
        if max_count_by_spk[spk]:
            runs.append((spk, max_count_by_spk[spk]))
    runs.append((0, max_linear))
    total = sum(n for _, n in runs)
    ng = (total + 127) // 128
    group_spk = []
    row = 0
    for g in range(ng):
        lo, hi = g * 128, min((g + 1) * 128, total)
        spk_g = 0
        r = 0
        for spk, n in runs:
            if r < hi and r + n > lo:
                spk_g = max(spk_g, spk)
            r += n
        group_spk.append(spk_g)
    return runs, group_spk


# ----------------------------------------------------------------------------
# per-core packing
# ----------------------------------------------------------------------------

def pack_core(keys_in_b, queries_b, values_b, pwl, structure):
    ts, ds, al = pwl
    runs, group_spk = structure
    ng = len(group_spk)
    kc = sum(group_spk)               # total knot columns
    nd = sum(s + 1 for s in group_spk)  # total diag blocks
    ch = keys_in_b[:, 0].astype(np.int32)
    pos = keys_in_b[:, 1].astype(np.float32)
    vsel = values_b[np.arange(K), ch].astype(np.float32)
    spk_of_key = np.array([len(ts[c]) - 1 for c in ch], np.int32)

    qrow = queries_b[:, 0].astype(np.float32)[None, :]
    posq = np.full((128, ng), DEAD_POS, np.float32)
    alphaq = np.zeros((128, ng), np.float32)
    knotq = np.full((128, kc), -9.0, np.float32)
    ddiag = np.zeros((128, 128 * nd), np.float16)
    ohov = np.zeros((128, 48 * ng), np.float16)

    kcol0 = np.concatenate([[0], np.cumsum(group_spk)])[:ng]
    dcol0 = np.concatenate([[0], np.cumsum([s + 1 for s in group_spk])])[:ng]

    # assign keys to rows: class-sorted (desc spk), then linear fills gaps
    by_spk = {}
    for k in range(K):
        by_spk.setdefault(int(spk_of_key[k]), []).append(k)
    linear_keys = by_spk.pop(0, [])
    row_iter = 0
    placements = []  # (key, global_row)
    for spk, nmax in runs:
        if spk == 0:
            continue
        pool = by_spk.pop(spk, [])
        assert len(pool) <= nmax, (spk, len(pool), nmax)
        for i, k in enumerate(pool):
            placements.append((k, row_iter + i))
        row_iter += nmax
    assert not by_spk, by_spk
    # linear keys: any remaining rows
    used = {r for _, r in placements}
    free = [r for r in range(ng * 128) if r not in used]
    assert len(free) >= len(linear_keys), (len(free), len(linear_keys))
    for k, r in zip(linear_keys, free):
        placements.append((k, r))

    for k, r in placements:
        g, row = r // 128, r % 128
        c = ch[k]
        posq[row, g] = pos[k]
        alphaq[row, g] = al[c]
        ohov[row, 48 * g + c] = np.float16(1.0)
        ohov[row, 48 * g + 32 + c] = np.float16(vsel[k])
        # beta diag block
        ddiag[row, 128 * dcol0[g] + row] = np.float16(ds[c][0])
        spk = len(ts[c]) - 1
        for j in range(spk):
            knotq[row, kcol0[g] + j] = np.float32(-ts[c][1 + j])
            ddiag[row, 128 * (dcol0[g] + 1 + j) + row] = np.float16(ds[c][1 + j])

    return dict(qrow=qrow, posq=posq, alphaq=alphaq, knotq=knotq,
                ddiag=ddiag, ohov=ohov)


# ----------------------------------------------------------------------------
# device program
# ----------------------------------------------------------------------------

def _build_program(structure):
    runs, group_spk = structure
    ng = len(group_spk)
    kc = sum(group_spk)
    nd = sum(s + 1 for s in group_spk)
    kcol0 = np.concatenate([[0], np.cumsum(group_spk)])[:ng]
    dcol0 = np.concatenate([[0], np.cumsum([s + 1 for s in group_spk])])[:ng]

    nc = bacc.Bacc("TRN2", target_bir_lowering=False, debug=False)

    d_qrow = nc.dram_tensor("qrow", [1, Q], F32, kind="ExternalInput")
    d_posq = nc.dram_tensor("posq", [128, ng], F32, kind="ExternalInput")
    d_alphaq = nc.dram_tensor("alphaq", [128, ng], F32, kind="ExternalInput")
    d_knotq = nc.dram_tensor("knotq", [128, max(kc, 1)], F32, kind="ExternalInput")
    d_ddiag = nc.dram_tensor("ddiag", [128, 128 * nd], F16, kind="ExternalInput")
    d_ohov = nc.dram_tensor("ohov", [128, 48 * ng], F16, kind="ExternalInput")
    d_sigp = nc.dram_tensor("sigp", [16, 2], F32, kind="ExternalInput")
    d_wrT = nc.dram_tensor("wrT", [16, 64], F32, kind="ExternalInput")
    d_brq = nc.dram_tensor("brq", [32, 1], F32, kind="ExternalInput")
    d_out = nc.dram_tensor("out", [32, Q], F32, kind="ExternalOutput")

    with tile.TileContext(nc) as tc:
        with tc.tile_pool(name="params", bufs=1) as params, \
             tc.tile_pool(name="qrep_p", bufs=1) as qrep_pool, \
             tc.tile_pool(name="a16_p", bufs=2) as a16_pool, \
             tc.tile_pool(name="a32_p", bufs=2) as a32_pool, \
             tc.tile_pool(name="u16_p", bufs=4) as u16_pool, \
             tc.tile_pool(name="wt_p", bufs=2) as wt_pool, \
             tc.tile_pool(name="w_p", bufs=3) as w_pool, \
             tc.tile_pool(name="epi_p", bufs=2) as epi_pool, \
             tc.tile_pool(name="qps", bufs=1, space="PSUM") as qps_pool, \
             tc.tile_pool(name="wps", bufs=4, space="PSUM") as wps_pool, \
             tc.tile_pool(name="dps", bufs=2, space="PSUM") as dps_pool:
            # PSUM banks: qps 1x[128,1024]=2 + wps 4x[128,512]=4 +
            #             dps 2x[48,512]=2  -> 8

            # hot-start params first, on the sync queue
            qrow_sb = params.tile([1, Q], F32, tag="qrow")
            nc.sync.dma_start(out=qrow_sb[:], in_=d_qrow.ap())
            posq_sb = params.tile([128, ng], F32, tag="posq")
            nc.sync.dma_start(out=posq_sb[:], in_=d_posq.ap())
            knotq_sb = params.tile([128, max(kc, 1)], F32, tag="knotq")
            nc.sync.dma_start(out=knotq_sb[:], in_=d_knotq.ap())
            alphaq_sb = params.tile([128, ng], F32, tag="alphaq")
            nc.sync.dma_start(out=alphaq_sb[:], in_=d_alphaq.ap())
            ohov_sb = params.tile([128, 48 * ng], F16, tag="ohov")
            nc.sync.dma_start(out=ohov_sb[:], in_=d_ohov.ap())
            # big diag tensor: per-group slices on the gpsimd queue so group 0
            # can start before the whole tensor lands
            ddiag_sb = params.tile([128, 128 * nd], F16, tag="ddiag")
            for g in range(ng):
                lo = 128 * dcol0[g]
                hi = 128 * (dcol0[g + 1] if g + 1 < ng else nd)
                nc.gpsimd.dma_start(out=ddiag_sb[:, lo:hi],
                                    in_=d_ddiag.ap()[:, lo:hi])
            sigp_sb = params.tile([16, 2], F32, tag="sigp")
            nc.sync.dma_start(out=sigp_sb[:], in_=d_sigp.ap())
            wrT_sb = params.tile([16, 64], F32, tag="wrT")
            nc.sync.dma_start(out=wrT_sb[:], in_=d_wrT.ap())
            brq_sb = params.tile([32, 1], F32, tag="brq")
            nc.sync.dma_start(out=brq_sb[:], in_=d_brq.ap())

            ones_sb = params.tile([1, 128], F32, tag="ones")
            nc.gpsimd.memset(ones_sb[:], 1.0)

            # Qrep for the whole batch: [128, 1024]
            qrep_ps = qps_pool.tile([128, Q], F32, tag="qps")
            for qt in range(NQT):
                nc.tensor.matmul(qrep_ps[:, qt * QT:(qt + 1) * QT], lhsT=ones_sb[:],
                                 rhs=qrow_sb[:, qt * QT:(qt + 1) * QT],
                                 start=True, stop=True)
            qrep = qrep_pool.tile([128, Q], F32, tag="qrep")
            nc.scalar.copy(qrep[:], qrep_ps[:])

            dt_ps = [dps_pool.tile([48, QT], F32, tag="dt", name=f"dt_ps{qt}")
                     for qt in range(NQT)]

            for g in range(ng):
                spk_g = group_spk[g]
                a32 = a32_pool.tile([128, Q], F32, tag="a32")
                nc.scalar.activation(a32[:], qrep[:], AF.Abs,
                                     bias=posq_sb[:, g:g + 1], scale=-1.0)
                a16 = a16_pool.tile([128, Q], F16, tag="a16")
                nc.vector.tensor_copy(a16[:], a32[:])

                w_ps = [wps_pool.tile([128, QT], F32, tag="wps",
                                      name=f"w_ps_g{g}q{qt}")
                        for qt in range(NQT)]
                db = int(dcol0[g])
                for qt in range(NQT):
                    nc.tensor.matmul(w_ps[qt][:],
                                     lhsT=ddiag_sb[:, 128 * db:128 * (db + 1)],
                                     rhs=a16[:, qt * QT:(qt + 1) * QT],
                                     start=True, stop=(spk_g == 0))
                for j in range(spk_g):
                    u16 = u16_pool.tile([128, Q], F16, tag="u16")
                    kcol = int(kcol0[g]) + j
                    nc.vector.tensor_scalar(u16[:], a16[:],
                                            knotq_sb[:, kcol:kcol + 1], 0.0,
                                            ALU.add, ALU.max)
                    for qt in range(NQT):
                        nc.tensor.matmul(
                            w_ps[qt][:],
                            lhsT=ddiag_sb[:, 128 * (db + 1 + j):128 * (db + 2 + j)],
                            rhs=u16[:, qt * QT:(qt + 1) * QT],
                            start=False, stop=(j == spk_g - 1))

                for qt in range(NQT):
                    wt16 = wt_pool.tile([128, QT], F16, tag="wt")
                    nc.scalar.activation(wt16[:], w_ps[qt][:], AF.Abs,
                                         bias=alphaq_sb[:, g:g + 1])
                    w16 = w_pool.tile([128, QT], F16, tag="w")
                    nc.vector.scalar_tensor_tensor(
                        w16[:], a32[:, qt * QT:(qt + 1) * QT], WINDOW, wt16[:],
                        ALU.is_lt, ALU.mult)
                    nc.tensor.matmul(dt_ps[qt][:],
                                     lhsT=ohov_sb[:, 48 * g:48 * (g + 1)],
                                     rhs=w16[:], start=(g == 0), stop=(g == ng - 1))

            for qt in range(NQT):
                qs = qt * QT
                den_sb = epi_pool.tile([16, QT], F32, tag="den_sb")
                nc.vector.tensor_scalar(den_sb[:], dt_ps[qt][0:16, :], 1e-5,
                                        None, ALU.add)
                rec = epi_pool.tile([16, QT], F32, tag="rec")
                scr = epi_pool.tile([16, QT], F32, tag="scr")
                nc.vector.reciprocal_approx_accurate(rec[:], den_sb[:], scr[:])
                tnum_sb = epi_pool.tile([16, QT], F32, tag="tnum_sb")
                nc.scalar.copy(tnum_sb[:], dt_ps[qt][32:48, :])

                targets = epi_pool.tile([16, QT], F32, tag="targets")
                nc.vector.tensor_tensor(targets[:], tnum_sb[:], rec[:], ALU.mult)
                dens = epi_pool.tile([16, QT], F32, tag="dens")
                nc.scalar.activation(dens[:], dt_ps[qt][0:16, :], AF.Sigmoid,
                                     bias=sigp_sb[:, 1:2], scale=sigp_sb[:, 0:1])

                out_ps = dps_pool.tile([32, QT], F32, tag="dt")
                nc.tensor.matmul(out_ps[:], lhsT=wrT_sb[:, 0:32], rhs=targets[:],
                                 start=True, stop=False)
                nc.tensor.matmul(out_ps[:], lhsT=wrT_sb[:, 32:64], rhs=dens[:],
                                 start=False, stop=True)
                out_sb = epi_pool.tile([32, QT], F32, tag="out_sb")
                nc.scalar.activation(out_sb[:], out_ps[:], AF.Identity,
                                     bias=brq_sb[:])
                nc.sync.dma_start(out=d_out.ap()[:, qs:qs + QT], in_=out_sb[:])

    nc.compile()
    return nc


_PROGRAM_CACHE = {}

LAST_EXEC_TIME_NS = None
LAST_RESULTS = None


def _ensure_ntff_hook():
    """The agent image's antenv lacks axon_hooks; synthesize it so
    run_bass_kernel_spmd(trace=True) can NTFF-profile via libaxon_pjrt.so."""
    import sys
    import types
    import ctypes
    import contextlib
    try:
        import antenv.axon_hooks  # noqa: F401
        return True
    except ImportError:
        pass
    so_path = "/opt/axon/libaxon_pjrt.so"
    try:
        lib = ctypes.CDLL(so_path)
    except OSError:
        return False
    if not hasattr(lib, "axon_start_nrt_profile"):
        return False
    lib.axon_start_nrt_profile.argtypes = [ctypes.POINTER(ctypes.c_int64),
                                           ctypes.c_size_t]
    lib.axon_start_nrt_profile.restype = ctypes.c_int64
    lib.axon_stop_nrt_profile.argtypes = [ctypes.c_char_p]
    lib.axon_stop_nrt_profile.restype = ctypes.c_int64

    @contextlib.contextmanager
    def _hook(output_dir, device_ids):
        import jax
        jax.devices()
        if device_ids:
            ids = (ctypes.c_int64 * len(device_ids))(*device_ids)
            rc = lib.axon_start_nrt_profile(ids, len(device_ids))
        else:
            rc = lib.axon_start_nrt_profile(None, 0)
        if rc != 0:
            raise RuntimeError(f"axon_start_nrt_profile rc={rc}")
        try:
            yield
        finally:
            n = lib.axon_stop_nrt_profile(str(output_dir).encode())
            print(f"profile: {n} file(s) written to {output_dir}")

    mod = types.ModuleType("antenv.axon_hooks")
    mod.get_axon_ntff_profile_hook = lambda: _hook
    mod.set_axon_ntff_profile_hook = lambda h: None
    import antenv
    antenv.axon_hooks = mod
    sys.modules["antenv.axon_hooks"] = mod
    return True


def _structure_key(structure):
    runs, group_spk = structure
    return (tuple(runs), tuple(group_spk))


def _get_program(structure):
    key = _structure_key(structure)
    if key not in _PROGRAM_CACHE:
        _PROGRAM_CACHE[key] = _build_program(structure)
    return _PROGRAM_CACHE[key]


# ----------------------------------------------------------------------------
# entry point
# ----------------------------------------------------------------------------

def kernel(trace=False, **inputs):
    global LAST_EXEC_TIME_NS, LAST_RESULTS
    keys_in = np.asarray(inputs["keys_in"], np.float32)
    queries = np.asarray(inputs["queries"], np.float32)
    values = np.asarray(inputs["values"], np.float32)
    W = {k: np.asarray(inputs[k], np.float32)
         for k in ["W0", "b0", "W1", "b1", "W2", "b2", "W3", "b3",
                   "Wd", "bd", "Wr", "br"]}

    pwl = _all_pwl(W["W0"], W["b0"], W["W1"], W["b1"], W["W2"], W["b2"],
                   W["W3"], W["b3"])
    ts = pwl[0]
    spk_by_channel = np.array([len(t) - 1 for t in ts], np.int32)

    # max #keys of each class over cores (spk==0 -> linear, no chunk needed)
    max_count = {}
    max_linear = 0
    for b in range(B):
        ch = keys_in[b, :, 0].astype(np.int32)
        spk = spk_by_channel[ch]
        max_linear = max(max_linear, int((spk == 0).sum()))
        for s in range(1, 17):
            n = int((spk == s).sum())
            if n:
                max_count[s] = max(max_count.get(s, 0), n)
    structure = plan_structure(max_count, max_linear)

    sig_scale = np.float32(0.1) * W["Wd"][0, 0]
    sig_bias = W["bd"][0] - W["Wd"][0, 0]
    sigp = np.stack([np.full(16, sig_scale, np.float32),
                     np.full(16, sig_bias, np.float32)], axis=1)
    Wr = W["Wr"]
    wrT = np.concatenate([Wr[:, :16].T, Wr[:, 16:].T], axis=1).astype(np.float32)
    brq = W["br"].astype(np.float32)[:, None]

    in_maps = []
    for b in range(B):
        packed = pack_core(keys_in[b], queries[b], values[b], pwl, structure)
        packed.update(sigp=sigp, wrT=wrT, brq=brq)
        in_maps.append(packed)

    nc = _get_program(structure)
    if trace:
        trace = _ensure_ntff_hook()
    res = run_bass_kernel_spmd(nc, in_maps, list(range(N_CORES)), trace=trace)
    LAST_RESULTS = res
    if trace:
        LAST_EXEC_TIME_NS = res.exec_time_ns
    out = np.stack([np.ascontiguousarray(res.results[i]["out"].T)
                    for i in range(N_CORES)], axis=0)
    return out.astype(np.float32)
